# revision 2
# baseline (speedup 1.0000x reference)
"""Trainium2 Bass kernel for nn_PointerLayer (fused QK projection + RoPE +
masked causal logits).

Sharding: 8 cores = 4 batches x 2 head-groups (6 heads each). Each core:
  projT = (W_perm @ x^T) per head      [128 rows = qe,qo,ke,ko | 1024 tokens]
  rope via two projections (plain + pair-swapped weights) and fused
  scalar_tensor_tensor ops with mask/scale-folded cos/sin tables,
  logits tile matmuls with 6 extra contraction rows carrying the exact
  -INF/8 mask terms, tril handled by K-row split + fused diagonal add.
"""

import sys

for _p in ("/opt/trn_rl_repo",):
    if _p not in sys.path:
        sys.path.append(_p)

import numpy as np

import concourse.bacc as bacc
import concourse.mybir as mybir
import concourse.tile as tile
from concourse.bass_utils import run_bass_kernel_spmd

F32 = mybir.dt.float32
F32R = mybir.dt.float32r
ADD = mybir.AluOpType.add
MULT = mybir.AluOpType.mult

B, N, HID = 4, 1024, 1024
HEADS, D = 12, 64
HPC = 6            # heads per core
N_CORES = 8
INF = 1000000000000.0
INF8 = np.float32(INF) / np.float32(8.0)      # = 124999999488.0, exact fp32
# exact 12-bit-significand factorization: (4206 + 1) * (3627 * 2^13) = INF8
C1A = 4206.0
C2 = 3627.0 * 8192.0

_CACHE = {}


def _build_program():
    nc = bacc.Bacc("TRN2", target_bir_lowering=False, debug=False,
                   num_devices=N_CORES)
    xt_d = nc.dram_tensor("xt", [8, 128, N], F32R, kind="ExternalInput").ap()
    w1_d = nc.dram_tensor("w1", [8, 128, HPC * 128], F32R, kind="ExternalInput").ap()
    cc_d = nc.dram_tensor("cc", [128, N], F32, kind="ExternalInput").ap()
    ss_d = nc.dram_tensor("ss", [128, N], F32, kind="ExternalInput").ap()
    b1_d = nc.dram_tensor("b1", [128, HPC], F32, kind="ExternalInput").ap()
    b2_d = nc.dram_tensor("b2", [128, HPC], F32, kind="ExternalInput").ap()
    mq_d = nc.dram_tensor("mq6", [6, N], F32R, kind="ExternalInput").ap()
    mk_d = nc.dram_tensor("mk6", [6, N], F32R, kind="ExternalInput").ap()
    tr_d = nc.dram_tensor("trl", [128, 128], F32, kind="ExternalInput").ap()
    out_d = nc.dram_tensor("out", [HPC, N, N], F32, kind="ExternalOutput").ap()

    with tile.TileContext(nc) as tc:
        with (
            tc.tile_pool(name="const", bufs=1) as cpool,
            tc.tile_pool(name="work", bufs=2) as tpool,
            tc.tile_pool(name="ext", bufs=2) as xpool,
            tc.tile_pool(name="outs", bufs=6) as opool,
            tc.tile_pool(name="pp", bufs=1, space="PSUM") as pppool,
            tc.tile_pool(name="po", bufs=4, space="PSUM") as popool,
        ):
            # ---- persistent loads ----
            xts, w1s, w2s = [], [], []
            for c in range(8):
                xtc = cpool.tile([128, N], F32R, tag=f"xt{c}")
                w1c = cpool.tile([128, HPC * 128], F32R, tag=f"w1{c}")
                nc.sync.dma_start(xtc[:], xt_d[c])
                nc.sync.dma_start(w1c[:], w1_d[c])
                xts.append(xtc)
                w1s.append(w1c)
            cc = cpool.tile([128, N], F32, tag="cc")
            ss = cpool.tile([128, N], F32, tag="ss")
            b1 = cpool.tile([128, HPC], F32, tag="b1")
            b2 = cpool.tile([128, HPC], F32, tag="b2")
            mq6 = cpool.tile([6, N], F32R, tag="mq6")
            mk6 = cpool.tile([6, N], F32R, tag="mk6")
            trl = cpool.tile([128, 128], F32, tag="trl")
            for t, d in ((cc, cc_d), (ss, ss_d), (b1, b1_d), (b2, b2_d),
                         (mq6, mq_d), (mk6, mk_d), (trl, tr_d)):
                nc.sync.dma_start(t[:], d[:])
            # swapped-weight copies (qe,qo,ke,ko) -> (qo,qe,ko,ke) per head
            for c in range(8):
                w2c = cpool.tile([128, HPC * 128], F32R, tag=f"w2{c}")
                src = w1s[c][:].rearrange("p (h q e i) -> p h q e i",
                                          h=HPC, q=2, e=2)
                dst = w2c[:].rearrange("p (h q e i) -> p h q e i",
                                       h=HPC, q=2, e=2)
                nc.vector.tensor_copy(dst, src[:, :, :, ::-1, :])
                w2s.append(w2c)

            # ---- per-head pipeline ----
            for h in range(HPC):
                hs = h * 128
                pp1 = pppool.tile([128, N], F32, tag="pp1")
                pp2 = pppool.tile([128, N], F32, tag="pp2")
                for c in range(8):
                    for cs in (0, 512):
                        nc.tensor.matmul(pp1[:, cs:cs + 512],
                                         w1s[c][:, hs:hs + 128],
                                         xts[c][:, cs:cs + 512],
                                         start=(c == 0), stop=(c == 7))
                        nc.tensor.matmul(pp2[:, cs:cs + 512],
                                         w2s[c][:, hs:hs + 128],
                                         xts[c][:, cs:cs + 512],
                                         start=(c == 0), stop=(c == 7))
                t1 = tpool.tile([128, N], F32, tag="t1")
                t2 = tpool.tile([128, N], F32, tag="t2")
                nc.vector.scalar_tensor_tensor(t1[:], pp1[:], b1[:, h:h + 1],
                                               cc[:], op0=ADD, op1=MULT)
                nc.vector.scalar_tensor_tensor(t2[:], pp2[:], b2[:, h:h + 1],
                                               ss[:], op0=ADD, op1=MULT)
                qx = xpool.tile([70, N], F32R, tag="qx")
                kx = xpool.tile([70, N], F32R, tag="kx")
                nc.sync.dma_start(qx[64:70, :], mq6[:])
                nc.sync.dma_start(kx[64:70, :], mk6[:])
                nc.vector.tensor_add(qx[0:64, :], t1[0:64, :], t2[0:64, :])
                nc.vector.tensor_add(kx[0:64, :], t1[64:128, :], t2[64:128, :])

                for mi in range(8):
                    bnd = 128 * mi
                    q70 = qx[0:70, bnd:bnd + 128]
                    q68 = qx[0:68, bnd:bnd + 128]
                    for cs in (0, 512):
                        ce = cs + 512
                        po = popool.tile([128, 512], F32, tag="po")
                        regions = []
                        if bnd > cs:            # strictly-below-diag columns
                            regions.append((cs, min(bnd, ce), 70))
                        if bnd < ce:            # diag block + above columns
                            regions.append((max(bnd, cs), ce, 68))
                        for ri, (rs, re, k) in enumerate(regions):
                            nc.tensor.matmul(
                                po[:, rs - cs:re - cs],
                                q70 if k == 70 else q68,
                                kx[0:k, rs:re],
                                start=(ri == 0), stop=(ri == len(regions) - 1))
                        ot = opool.tile([128, 512], F32, tag="ot")
                        # psum -> sbuf, fusing the tril constant on the
                        # diagonal block; balance plain copies DVE/ACT
                        segs = []   # (off, width, is_diag)
                        if cs <= bnd < ce:
                            dof = bnd - cs
                            if dof > 0:
                                segs.append((0, dof, False))
                            segs.append((dof, 128, True))
                            if dof + 128 < 512:
                                segs.append((dof + 128, 512 - dof - 128, False))
                        else:
                            segs.append((0, 512, False))
                        dve_cols = 128 * sum(1 for s in segs if s[2])
                        for off, wd, isdiag in segs:
                            if isdiag:
                                nc.vector.tensor_add(ot[:, off:off + wd],
                                                     po[:, off:off + wd],
                                                     trl[:])
                            else:
                                # give DVE ~256 cols/chunk total, ACT the rest
                                take = max(0, min(wd, 256 - dve_cols))
                                if take > 0:
                                    nc.vector.tensor_copy(
                                        ot[:, off:off + take],
                                        po[:, off:off + take])
                                    dve_cols += take
                                if take < wd:
                                    nc.scalar.copy(ot[:, off + take:off + wd],
                                                   po[:, off + take:off + wd])
                        nc.sync.dma_start(
                            out_d[h, bnd:bnd + 128, cs:ce], ot[:])
    nc.compile()
    return nc


def _rope_tables():
    pos = np.arange(N, dtype=np.float32)
    inv = np.power(np.float32(10000.0),
                   np.float32(-2.0) * np.arange(D // 2, dtype=np.float32)
                   / np.float32(D))
    ang = pos[:, None] * inv[None, :]          # [N, 32] fp32
    return np.sin(ang).T.copy(), np.cos(ang).T.copy()   # [32, N]


def _perm(deinterleave_swap):
    # per-head row order: qe,qo,ke,ko (or qo,qe,ko,ke when swapped)
    ev, od = np.arange(0, 64, 2), np.arange(1, 64, 2)
    if deinterleave_swap:
        blocks = [od, ev, 64 + od, 64 + ev]
    else:
        blocks = [ev, od, 64 + ev, 64 + od]
    one = np.concatenate(blocks)
    return np.concatenate([h * 128 + one for h in range(HPC)])


def _prepare_in_maps(context_output, mask, W, b):
    x = np.asarray(context_output, np.float32)
    mask = np.asarray(mask, np.float32)
    W = np.asarray(W, np.float32)
    b = np.asarray(b, np.float32)

    sinT, cosT = _rope_tables()                 # [32, N] fp32
    p1, p2 = _perm(False), _perm(True)
    tril = np.where(np.arange(128)[:, None] > np.arange(128)[None, :],
                    np.float32(-INF8), np.float32(0.0)).astype(np.float32)

    per_g = {}
    for g in range(2):
        Wg = W[g * 768:(g + 1) * 768]
        bg = b[g * 768:(g + 1) * 768]
        w1 = np.ascontiguousarray(Wg[p1].T).reshape(8, 128, 768)
        b1 = np.ascontiguousarray(bg[p1].reshape(HPC, 128).T)
        b2 = np.ascontiguousarray(bg[p2].reshape(HPC, 128).T)
        per_g[g] = (w1, b1, b2)

    in_maps = []
    for c in range(N_CORES):
        bi, g = c // 2, c % 2
        w1, b1, b2 = per_g[g]
        xt = np.ascontiguousarray(x[bi].T).reshape(8, 128, N)
        m = mask[bi]
        mq8 = (m / np.float32(8.0)).astype(np.float32)
        cq, sq = cosT * mq8, sinT * mq8
        ck, sk = cosT * m, sinT * m
        cc = np.concatenate([cq, cq, ck, ck], 0).astype(np.float32)
        ss = np.concatenate([-sq, sq, -sk, sk], 0).astype(np.float32)
        im = np.float32(1.0) - m
        mq6 = np.stack([im * np.float32(C1A), im,
                        m * np.float32(C1A), m,
                        np.full(N, C1A, np.float32),
                        np.ones(N, np.float32)]).astype(np.float32)
        nk = np.full(N, np.float32(-C2), np.float32)
        mk6 = np.stack([nk, nk, im * np.float32(-C2), im * np.float32(-C2),
                        nk, nk]).astype(np.float32)
        in_maps.append({
            "xt": xt, "w1": w1, "cc": cc, "ss": ss, "b1": b1, "b2": b2,
            "mq6": mq6, "mk6": mk6, "trl": tril,
        })
    return in_maps


def _get_nc():
    if "nc" not in _CACHE:
        _CACHE["nc"] = _build_program()
    return _CACHE["nc"]


def _get_runner():
    """Build (once) a jitted shard_map executor over the 8 cores, mirroring
    bass_utils.run_bass_kernel_spmd's axon path but reusable across calls."""
    if "runner" in _CACHE:
        return _CACHE["runner"]
    import jax
    from jax.sharding import Mesh, NamedSharding, PartitionSpec
    from jax.experimental.shard_map import shard_map
    import concourse.bass2jax as bass2jax

    nc = _get_nc()
    bass2jax.install_neuronx_cc_hook()
    partition_name = (nc.partition_id_tensor.name
                      if nc.partition_id_tensor else None)
    in_names, out_names, out_avals = [], [], []
    for alloc in nc.m.functions[0].allocations:
        if not isinstance(alloc, mybir.MemoryLocationSet):
            continue
        name = alloc.memorylocations[0].name
        if alloc.kind == "ExternalInput":
            if name != partition_name:
                in_names.append(name)
        elif alloc.kind == "ExternalOutput":
            out_names.append(name)
            out_avals.append(jax.core.ShapedArray(
                tuple(alloc.tensor_shape), mybir.dt.np(alloc.dtype)))
    n_params = len(in_names)
    all_names = in_names + out_names
    if partition_name is not None:
        all_names = all_names + [partition_name]

    def _body(*args):
        operands = list(args)
        if partition_name is not None:
            operands.append(bass2jax.partition_id_tensor())
        outs = bass2jax._bass_exec_p.bind(
            *operands, out_avals=tuple(out_avals), in_names=tuple(all_names),
            out_names=tuple(out_names), lowering_input_output_aliases=(),
            sim_require_finite=True, sim_require_nnan=True, nc=nc)
        return tuple(outs)

    devices = jax.devices()[:N_CORES]
    mesh = Mesh(np.asarray(devices), ("core",))
    nin = n_params + len(out_names)
    sharded = jax.jit(
        shard_map(_body, mesh=mesh,
                  in_specs=(PartitionSpec("core"),) * nin,
                  out_specs=(PartitionSpec("core"),) * len(out_names),
                  check_rep=False),
        donate_argnums=tuple(range(n_params, nin)), keep_unused=True)
    shard = NamedSharding(mesh, PartitionSpec("core"))
    zero_shapes = [(N_CORES * a.shape[0], *a.shape[1:]) for a in out_avals]
    runner = {
        "sharded": sharded, "shard": shard, "in_names": in_names,
        "out_names": out_names, "out_avals": out_avals,
        "zero_shapes": zero_shapes,
    }
    _CACHE["runner"] = runner
    return runner


def _run(in_maps):
    import jax
    r = _get_runner()
    concat_in = [np.concatenate([in_maps[c][n] for c in range(N_CORES)], 0)
                 for n in r["in_names"]]
    dev_in = [jax.device_put(a, r["shard"]) for a in concat_in]
    zeros = [jax.device_put(np.zeros(s, np.float32), r["shard"])
             for s in r["zero_shapes"]]
    outs = r["sharded"](*dev_in, *zeros)
    return [
        {n: np.asarray(outs[i]).reshape(N_CORES, *r["out_avals"][i].shape)[c]
         for i, n in enumerate(r["out_names"])}
        for c in range(N_CORES)
    ]


def kernel(context_output, mask, W, b):
    in_maps = _prepare_in_maps(context_output, mask, W, b)
    results = _run(in_maps)
    out = np.empty((B, HEADS, N, N), np.float32)
    for c in range(N_CORES):
        bi, g = c // 2, c % 2
        out[bi, g * HPC:(g + 1) * HPC] = results[c]["out"]
    return out


# revision 7
# speedup vs baseline: 729.1286x; 729.1286x over previous
"""Trainium2 Bass kernel for nn_PointerLayer (fused QK projection + RoPE +
masked causal logits).

Sharding: 8 cores = 4 batches x 2 head-groups (6 heads each). Each core:
  projT = (W_perm @ x^T) per head      [128 rows = qe,qo,ke,ko | 1024 tokens]
  rope via two projections (plain + pair-swapped weights) and fused
  scalar_tensor_tensor ops with mask/scale-folded cos/sin tables,
  logits tile matmuls with 6 extra contraction rows carrying the exact
  -INF/8 mask terms, tril handled by K-row split + fused diagonal add.
"""

import sys

for _p in ("/opt/trn_rl_repo",):
    if _p not in sys.path:
        sys.path.append(_p)

import numpy as np

import concourse.bacc as bacc
import concourse.mybir as mybir
import concourse.tile as tile
from concourse.bass_utils import run_bass_kernel_spmd

F32 = mybir.dt.float32
F32R = mybir.dt.float32r
ADD = mybir.AluOpType.add
MULT = mybir.AluOpType.mult

B, N, HID = 4, 1024, 1024
HEADS, D = 12, 64
HPC = 6            # heads per core
N_CORES = 8
INF = 1000000000000.0
INF8 = np.float32(INF) / np.float32(8.0)      # = 124999999488.0, exact fp32
# exact 12-bit-significand factorization: (4206 + 1) * (3627 * 2^13) = INF8
C1A = 4206.0
C2 = 3627.0 * 8192.0

_CACHE = {}


def _build_program(repeat=1):
    nc = bacc.Bacc("TRN2", target_bir_lowering=False, debug=False,
                   num_devices=N_CORES)
    xt_d = nc.dram_tensor("xt", [8, 128, N], F32R, kind="ExternalInput").ap()
    w1_d = nc.dram_tensor("w1", [8, 128, HPC * 128], F32R, kind="ExternalInput").ap()
    cc_d = nc.dram_tensor("cc", [128, N], F32, kind="ExternalInput").ap()
    ss_d = nc.dram_tensor("ss", [128, N], F32, kind="ExternalInput").ap()
    b1_d = nc.dram_tensor("b1", [128, HPC], F32, kind="ExternalInput").ap()
    b2_d = nc.dram_tensor("b2", [128, HPC], F32, kind="ExternalInput").ap()
    mq_d = nc.dram_tensor("mq6", [6, N], F32R, kind="ExternalInput").ap()
    mk_d = nc.dram_tensor("mk6", [6, N], F32R, kind="ExternalInput").ap()
    tr_d = nc.dram_tensor("trl", [128, 128], F32, kind="ExternalInput").ap()
    out_d = nc.dram_tensor("out", [HPC, N, N], F32, kind="ExternalOutput").ap()

    with tile.TileContext(nc) as tc:
        with (
            tc.tile_pool(name="const", bufs=1) as cpool,
            tc.tile_pool(name="work", bufs=2) as tpool,
            tc.tile_pool(name="ext", bufs=2) as xpool,
            tc.tile_pool(name="outs", bufs=6) as opool,
            tc.tile_pool(name="pp", bufs=1, space="PSUM") as pppool,
            tc.tile_pool(name="po", bufs=4, space="PSUM") as popool,
        ):
            # ---- persistent loads ----
            xts, w1s, w2s = [], [], []
            for c in range(8):
                xtc = cpool.tile([128, N], F32R, tag=f"xt{c}")
                w1c = cpool.tile([128, HPC * 128], F32R, tag=f"w1{c}")
                nc.sync.dma_start(xtc[:], xt_d[c])
                nc.sync.dma_start(w1c[:], w1_d[c])
                xts.append(xtc)
                w1s.append(w1c)
            cc = cpool.tile([128, N], F32, tag="cc")
            ss = cpool.tile([128, N], F32, tag="ss")
            b1 = cpool.tile([128, HPC], F32, tag="b1")
            b2 = cpool.tile([128, HPC], F32, tag="b2")
            mq6 = cpool.tile([6, N], F32R, tag="mq6")
            mk6 = cpool.tile([6, N], F32R, tag="mk6")
            trl = cpool.tile([128, 128], F32, tag="trl")
            for t, d in ((cc, cc_d), (ss, ss_d), (b1, b1_d), (b2, b2_d),
                         (mq6, mq_d), (mk6, mk_d), (trl, tr_d)):
                nc.sync.dma_start(t[:], d[:])
            # swapped-weight copies (qe,qo,ke,ko) -> (qo,qe,ko,ke) per head
            for c in range(8):
                w2c = cpool.tile([128, HPC * 128], F32R, tag=f"w2{c}")
                src = w1s[c][:].rearrange("p (h q e i) -> p h q e i",
                                          h=HPC, q=2, e=2)
                dst = w2c[:].rearrange("p (h q e i) -> p h q e i",
                                       h=HPC, q=2, e=2)
                nc.vector.tensor_copy(dst, src[:, :, :, ::-1, :])
                w2s.append(w2c)

            # ---- per-head pipeline ----
            def emit_body():
                _emit_heads(nc, tpool, xpool, opool, pppool, popool,
                            xts, w1s, w2s, cc, ss, b1, b2, mq6, mk6, trl,
                            out_d)

            if repeat > 1:
                with tc.For_i(0, repeat, 1):
                    emit_body()
            else:
                emit_body()
    nc.compile()
    return nc


def _emit_heads(nc, tpool, xpool, opool, pppool, popool,
                xts, w1s, w2s, cc, ss, b1, b2, mq6, mk6, trl, out_d):
            for h in range(HPC):
                hs = h * 128
                pp1 = pppool.tile([128, N], F32, tag="pp1")
                pp2 = pppool.tile([128, N], F32, tag="pp2")
                for c in range(8):
                    for cs in (0, 512):
                        nc.tensor.matmul(pp1[:, cs:cs + 512],
                                         w1s[c][:, hs:hs + 128],
                                         xts[c][:, cs:cs + 512],
                                         start=(c == 0), stop=(c == 7))
                        nc.tensor.matmul(pp2[:, cs:cs + 512],
                                         w2s[c][:, hs:hs + 128],
                                         xts[c][:, cs:cs + 512],
                                         start=(c == 0), stop=(c == 7))
                t1 = tpool.tile([128, N], F32, tag="t1")
                t2 = tpool.tile([128, N], F32, tag="t2")
                nc.vector.scalar_tensor_tensor(t1[:], pp1[:], b1[:, h:h + 1],
                                               cc[:], op0=ADD, op1=MULT)
                nc.vector.scalar_tensor_tensor(t2[:], pp2[:], b2[:, h:h + 1],
                                               ss[:], op0=ADD, op1=MULT)
                qx = xpool.tile([70, N], F32R, tag="qx")
                kx = xpool.tile([70, N], F32R, tag="kx")
                nc.sync.dma_start(qx[64:70, :], mq6[:])
                nc.sync.dma_start(kx[64:70, :], mk6[:])
                nc.vector.tensor_add(qx[0:64, :], t1[0:64, :], t2[0:64, :])
                nc.vector.tensor_add(kx[0:64, :], t1[64:128, :], t2[64:128, :])

                for mi in range(8):
                    bnd = 128 * mi
                    q70 = qx[0:70, bnd:bnd + 128]
                    q68 = qx[0:68, bnd:bnd + 128]
                    for cs in (0, 512):
                        ce = cs + 512
                        po = popool.tile([128, 512], F32, tag="po")
                        regions = []
                        if bnd > cs:            # strictly-below-diag columns
                            regions.append((cs, min(bnd, ce), 70))
                        if bnd < ce:            # diag block + above columns
                            regions.append((max(bnd, cs), ce, 68))
                        for ri, (rs, re, k) in enumerate(regions):
                            nc.tensor.matmul(
                                po[:, rs - cs:re - cs],
                                q70 if k == 70 else q68,
                                kx[0:k, rs:re],
                                start=(ri == 0), stop=(ri == len(regions) - 1))
                        ot = opool.tile([128, 512], F32, tag="ot")
                        # psum -> sbuf, fusing the tril constant on the
                        # diagonal block; balance plain copies DVE/ACT
                        segs = []   # (off, width, is_diag)
                        if cs <= bnd < ce:
                            dof = bnd - cs
                            if dof > 0:
                                segs.append((0, dof, False))
                            segs.append((dof, 128, True))
                            if dof + 128 < 512:
                                segs.append((dof + 128, 512 - dof - 128, False))
                        else:
                            segs.append((0, 512, False))
                        dve_cols = 128 * sum(1 for s in segs if s[2])
                        for off, wd, isdiag in segs:
                            if isdiag:
                                nc.vector.tensor_add(ot[:, off:off + wd],
                                                     po[:, off:off + wd],
                                                     trl[:])
                            else:
                                # give DVE ~256 cols/chunk total, ACT the rest
                                take = max(0, min(wd, 256 - dve_cols))
                                if take > 0:
                                    nc.vector.tensor_copy(
                                        ot[:, off:off + take],
                                        po[:, off:off + take])
                                    dve_cols += take
                                if take < wd:
                                    nc.scalar.copy(ot[:, off + take:off + wd],
                                                   po[:, off + take:off + wd])
                        nc.sync.dma_start(
                            out_d[h, bnd:bnd + 128, cs:ce], ot[:])


def _rope_tables():
    pos = np.arange(N, dtype=np.float32)
    inv = np.power(np.float32(10000.0),
                   np.float32(-2.0) * np.arange(D // 2, dtype=np.float32)
                   / np.float32(D))
    ang = pos[:, None] * inv[None, :]          # [N, 32] fp32
    return np.sin(ang).T.copy(), np.cos(ang).T.copy()   # [32, N]


def _perm(deinterleave_swap):
    # per-head row order: qe,qo,ke,ko (or qo,qe,ko,ke when swapped)
    ev, od = np.arange(0, 64, 2), np.arange(1, 64, 2)
    if deinterleave_swap:
        blocks = [od, ev, 64 + od, 64 + ev]
    else:
        blocks = [ev, od, 64 + ev, 64 + od]
    one = np.concatenate(blocks)
    return np.concatenate([h * 128 + one for h in range(HPC)])


def _prepare_in_maps(context_output, mask, W, b):
    x = np.asarray(context_output, np.float32)
    mask = np.asarray(mask, np.float32)
    W = np.asarray(W, np.float32)
    b = np.asarray(b, np.float32)

    sinT, cosT = _rope_tables()                 # [32, N] fp32
    p1, p2 = _perm(False), _perm(True)
    tril = np.where(np.arange(128)[:, None] > np.arange(128)[None, :],
                    np.float32(-INF8), np.float32(0.0)).astype(np.float32)

    per_g = {}
    for g in range(2):
        Wg = W[g * 768:(g + 1) * 768]
        bg = b[g * 768:(g + 1) * 768]
        w1 = np.ascontiguousarray(Wg[p1].T).reshape(8, 128, 768)
        b1 = np.ascontiguousarray(bg[p1].reshape(HPC, 128).T)
        b2 = np.ascontiguousarray(bg[p2].reshape(HPC, 128).T)
        per_g[g] = (w1, b1, b2)

    in_maps = []
    for c in range(N_CORES):
        bi, g = c // 2, c % 2
        w1, b1, b2 = per_g[g]
        xt = np.ascontiguousarray(x[bi].T).reshape(8, 128, N)
        m = mask[bi]
        mq8 = (m / np.float32(8.0)).astype(np.float32)
        cq, sq = cosT * mq8, sinT * mq8
        ck, sk = cosT * m, sinT * m
        cc = np.concatenate([cq, cq, ck, ck], 0).astype(np.float32)
        ss = np.concatenate([-sq, sq, -sk, sk], 0).astype(np.float32)
        im = np.float32(1.0) - m
        mq6 = np.stack([im * np.float32(C1A), im,
                        m * np.float32(C1A), m,
                        np.full(N, C1A, np.float32),
                        np.ones(N, np.float32)]).astype(np.float32)
        nk = np.full(N, np.float32(-C2), np.float32)
        mk6 = np.stack([nk, nk, im * np.float32(-C2), im * np.float32(-C2),
                        nk, nk]).astype(np.float32)
        in_maps.append({
            "xt": xt, "w1": w1, "cc": cc, "ss": ss, "b1": b1, "b2": b2,
            "mq6": mq6, "mk6": mk6, "trl": tril,
        })
    return in_maps


def _get_nc():
    if "nc" not in _CACHE:
        _CACHE["nc"] = _build_program()
    return _CACHE["nc"]


def _get_runner():
    if "runner" not in _CACHE:
        _CACHE["runner"] = _make_runner(_get_nc())
    return _CACHE["runner"]


def _make_runner(nc):
    """Build a jitted shard_map executor over the 8 cores, mirroring
    bass_utils.run_bass_kernel_spmd's axon path but reusable across calls."""
    import jax
    from jax.sharding import Mesh, NamedSharding, PartitionSpec
    from jax.experimental.shard_map import shard_map
    import concourse.bass2jax as bass2jax

    bass2jax.install_neuronx_cc_hook()
    partition_name = (nc.partition_id_tensor.name
                      if nc.partition_id_tensor else None)
    in_names, out_names, out_avals = [], [], []
    for alloc in nc.m.functions[0].allocations:
        if not isinstance(alloc, mybir.MemoryLocationSet):
            continue
        name = alloc.memorylocations[0].name
        if alloc.kind == "ExternalInput":
            if name != partition_name:
                in_names.append(name)
        elif alloc.kind == "ExternalOutput":
            out_names.append(name)
            out_avals.append(jax.core.ShapedArray(
                tuple(alloc.tensor_shape), mybir.dt.np(alloc.dtype)))
    n_params = len(in_names)
    all_names = in_names + out_names
    if partition_name is not None:
        all_names = all_names + [partition_name]

    def _body(*args):
        operands = list(args)
        if partition_name is not None:
            operands.append(bass2jax.partition_id_tensor())
        outs = bass2jax._bass_exec_p.bind(
            *operands, out_avals=tuple(out_avals), in_names=tuple(all_names),
            out_names=tuple(out_names), lowering_input_output_aliases=(),
            sim_require_finite=True, sim_require_nnan=True, nc=nc)
        return tuple(outs)

    devices = jax.devices()[:N_CORES]
    mesh = Mesh(np.asarray(devices), ("core",))
    nin = n_params + len(out_names)
    sharded = jax.jit(
        shard_map(_body, mesh=mesh,
                  in_specs=(PartitionSpec("core"),) * nin,
                  out_specs=(PartitionSpec("core"),) * len(out_names),
                  check_rep=False),
        donate_argnums=tuple(range(n_params, nin)), keep_unused=True)
    shard = NamedSharding(mesh, PartitionSpec("core"))
    zero_shapes = [(N_CORES * a.shape[0], *a.shape[1:]) for a in out_avals]
    return {
        "sharded": sharded, "shard": shard, "in_names": in_names,
        "out_names": out_names, "out_avals": out_avals,
        "zero_shapes": zero_shapes,
    }


def _run(in_maps):
    import jax
    r = _get_runner()
    concat_in = [np.concatenate([in_maps[c][n] for c in range(N_CORES)], 0)
                 for n in r["in_names"]]
    dev_in = [jax.device_put(a, r["shard"]) for a in concat_in]
    zeros = [jax.device_put(np.zeros(s, np.float32), r["shard"])
             for s in r["zero_shapes"]]
    outs = r["sharded"](*dev_in, *zeros)
    return [
        {n: np.asarray(outs[i]).reshape(N_CORES, *r["out_avals"][i].shape)[c]
         for i, n in enumerate(r["out_names"])}
        for c in range(N_CORES)
    ]


def kernel(context_output, mask, W, b):
    in_maps = _prepare_in_maps(context_output, mask, W, b)
    results = _run(in_maps)
    out = np.empty((B, HEADS, N, N), np.float32)
    for c in range(N_CORES):
        bi, g = c // 2, c % 2
        out[bi, g * HPC:(g + 1) * HPC] = results[c]["out"]
    return out


# revision 17
# speedup vs baseline: 1071.1086x; 1.4690x over previous
"""Trainium2 Bass kernel for nn_PointerLayer (fused QK projection + RoPE +
masked causal logits).

Sharding: 8 cores = 4 batches x 2 head-groups (6 heads each). Each core:
  projT = (W_perm @ x^T) per head      [128 rows = qe,qo,ke,ko | 1024 tokens]
  rope via two projections (plain + pair-swapped weights) and fused
  scalar_tensor_tensor ops with mask/scale-folded cos/sin tables,
  logits tile matmuls with 6 extra contraction rows carrying the exact
  -INF/8 mask terms, tril handled by K-row split + fused diagonal add.
"""

import sys

for _p in ("/opt/trn_rl_repo",):
    if _p not in sys.path:
        sys.path.append(_p)

import numpy as np

import concourse.bacc as bacc
import concourse.mybir as mybir
import concourse.tile as tile
from concourse.bass_utils import run_bass_kernel_spmd

F32 = mybir.dt.float32
F32R = mybir.dt.float32r
ADD = mybir.AluOpType.add
MULT = mybir.AluOpType.mult

B, N, HID = 4, 1024, 1024
HEADS, D = 12, 64
HPC = 6            # heads per core
N_CORES = 8
INF = 1000000000000.0
INF8 = np.float32(INF) / np.float32(8.0)      # = 124999999488.0, exact fp32
# exact 12-bit-significand factorization: (4206 + 1) * (3627 * 2^13) = INF8
C1A = 4206.0
C2 = 3627.0 * 8192.0

_CACHE = {}


def _build_program(repeat=1):
    nc = bacc.Bacc("TRN2", target_bir_lowering=False, debug=False,
                   num_devices=N_CORES)
    xt_d = nc.dram_tensor("xt", [8, 128, N], F32R, kind="ExternalInput").ap()
    w1_d = nc.dram_tensor("w1", [8, 128, HPC * 128], F32R, kind="ExternalInput").ap()
    cc_d = nc.dram_tensor("cc", [128, N], F32, kind="ExternalInput").ap()
    ss_d = nc.dram_tensor("ss", [128, N], F32, kind="ExternalInput").ap()
    b1_d = nc.dram_tensor("b1", [128, HPC], F32, kind="ExternalInput").ap()
    b2_d = nc.dram_tensor("b2", [128, HPC], F32, kind="ExternalInput").ap()
    mq_d = nc.dram_tensor("mq6", [6, N], F32R, kind="ExternalInput").ap()
    mk_d = nc.dram_tensor("mk6", [6, N], F32R, kind="ExternalInput").ap()
    tr_d = nc.dram_tensor("trl", [128, 128], F32, kind="ExternalInput").ap()
    out_d = nc.dram_tensor("out", [HPC, N, N], F32, kind="ExternalOutput").ap()

    with tile.TileContext(nc) as tc:
        with (
            tc.tile_pool(name="const", bufs=1) as cpool,
            tc.tile_pool(name="work", bufs=2) as tpool,
            tc.tile_pool(name="ext", bufs=2) as xpool,
            tc.tile_pool(name="outs", bufs=6) as opool,
            tc.tile_pool(name="pp", bufs=2, space="PSUM") as pppool,
            tc.tile_pool(name="po", bufs=4, space="PSUM") as popool,
        ):
            # ---- persistent loads ----
            xts, w1s = [], []
            for c in range(8):
                xtc = cpool.tile([128, N], F32R, tag=f"xt{c}")
                w1c = cpool.tile([128, HPC * 128], F32R, tag=f"w1{c}")
                nc.sync.dma_start(xtc[:], xt_d[c])
                nc.sync.dma_start(w1c[:], w1_d[c])
                xts.append(xtc)
                w1s.append(w1c)
            cc = cpool.tile([128, N], F32, tag="cc")
            ss = cpool.tile([128, N], F32, tag="ss")
            b1 = cpool.tile([128, HPC], F32, tag="b1")
            b2 = cpool.tile([128, HPC], F32, tag="b2")
            mq6 = cpool.tile([6, N], F32R, tag="mq6")
            mk6 = cpool.tile([6, N], F32R, tag="mk6")
            trl = cpool.tile([128, 128], F32, tag="trl")
            for t, d in ((cc, cc_d), (ss, ss_d), (b1, b1_d), (b2, b2_d),
                         (mq6, mq_d), (mk6, mk_d), (trl, tr_d)):
                nc.sync.dma_start(t[:], d[:])
            # ---- per-head pipeline ----
            def emit_body():
                _emit_heads(nc, tpool, xpool, opool, pppool, popool,
                            xts, w1s, cc, ss, b1, b2, mq6, mk6, trl,
                            out_d)

            if repeat > 1:
                with tc.For_i(0, repeat, 1):
                    emit_body()
            else:
                emit_body()
    nc.compile()
    return nc


def _emit_heads(nc, tpool, xpool, opool, pppool, popool,
                xts, w1s, cc, ss, b1, b2, mq6, mk6, trl, out_d):
            for h in range(HPC):
                hs = h * 128
                pp1 = pppool.tile([128, N], F32, tag="pp1")
                for c in range(8):
                    for cs in (0, 512):
                        nc.tensor.matmul(pp1[:, cs:cs + 512],
                                         w1s[c][:, hs:hs + 128],
                                         xts[c][:, cs:cs + 512],
                                         start=(c == 0), stop=(c == 7))
                # p2 = rope-partner rows of pp1: with the 16-interleaved
                # layout the partner sits +16 within each 32-partition
                # quadrant, so one stream_shuffle does the whole swap
                p2 = tpool.tile([128, N], F32, tag="p2")
                nc.vector.stream_shuffle(p2[:], pp1[:],
                                         [(i + 16) % 32 for i in range(32)])
                t1 = tpool.tile([128, N], F32, tag="t1")
                t2 = tpool.tile([128, N], F32, tag="t2")
                nc.vector.scalar_tensor_tensor(t1[:], pp1[:], b1[:, h:h + 1],
                                               cc[:], op0=ADD, op1=MULT)
                nc.vector.scalar_tensor_tensor(t2[:], p2[:], b2[:, h:h + 1],
                                               ss[:], op0=ADD, op1=MULT)
                qx = xpool.tile([70, N], F32R, tag="qx")
                kx = xpool.tile([70, N], F32R, tag="kx")
                nc.sync.dma_start(qx[64:70, :], mq6[:])
                nc.sync.dma_start(kx[64:70, :], mk6[:])
                nc.vector.tensor_add(qx[0:64, :], t1[0:64, :], t2[0:64, :])
                nc.gpsimd.tensor_add(kx[0:64, :], t1[64:128, :], t2[64:128, :])

                for mi in range(8):
                    bnd = 128 * mi
                    q70 = qx[0:70, bnd:bnd + 128]
                    q68 = qx[0:68, bnd:bnd + 128]
                    ot = opool.tile([128, N], F32, tag="ot")
                    for cs in (0, 512):
                        ce = cs + 512
                        po = popool.tile([128, 512], F32, tag="po")
                        regions = []
                        if bnd > cs:            # strictly-below-diag columns
                            regions.append((cs, min(bnd, ce), 70))
                        if bnd < ce:            # diag block + above columns
                            regions.append((max(bnd, cs), ce, 68))
                        for ri, (rs, re, k) in enumerate(regions):
                            nc.tensor.matmul(
                                po[:, rs - cs:re - cs],
                                q70 if k == 70 else q68,
                                kx[0:k, rs:re],
                                start=(ri == 0), stop=(ri == len(regions) - 1))
                        # psum -> sbuf, fusing the tril constant on the
                        # diagonal block; balance plain copies DVE/ACT
                        segs = []   # (off, width, is_diag)
                        if cs <= bnd < ce:
                            dof = bnd - cs
                            if dof > 0:
                                segs.append((0, dof, False))
                            segs.append((dof, 128, True))
                            if dof + 128 < 512:
                                segs.append((dof + 128, 512 - dof - 128, False))
                        else:
                            segs.append((0, 512, False))
                        dve_cols = 128 * sum(1 for s in segs if s[2])
                        for off, wd, isdiag in segs:
                            oo = cs + off
                            if isdiag:
                                nc.vector.tensor_add(ot[:, oo:oo + wd],
                                                     po[:, off:off + wd],
                                                     trl[:])
                            else:
                                # give DVE ~128 cols/chunk total, ACT the rest
                                take = max(0, min(wd, 128 - dve_cols))
                                if take > 0:
                                    nc.vector.tensor_copy(
                                        ot[:, oo:oo + take],
                                        po[:, off:off + take])
                                    dve_cols += take
                                if take < wd:
                                    nc.scalar.copy(ot[:, oo + take:oo + wd],
                                                   po[:, off + take:off + wd])
                    nc.sync.dma_start(out_d[h, bnd:bnd + 128, :], ot[:])


def _rope_tables():
    pos = np.arange(N, dtype=np.float32)
    inv = np.power(np.float32(10000.0),
                   np.float32(-2.0) * np.arange(D // 2, dtype=np.float32)
                   / np.float32(D))
    ang = pos[:, None] * inv[None, :]          # [N, 32] fp32
    return np.sin(ang).T.copy(), np.cos(ang).T.copy()   # [32, N]


def _perm16():
    """Per-head partition layout: within each 32-partition quadrant, rows
    0:16 hold even rope dims (x[2f]) and rows 16:32 their odd partners
    (x[2f+1]); quadrants 0,1 = q (f 0:16, 16:32), quadrants 2,3 = k.
    Returns perm[p] = source row within the head's 128 W rows."""
    f = np.arange(32)
    p_e = (f // 16) * 32 + (f % 16)
    p_o = p_e + 16
    perm = np.zeros(128, dtype=np.int64)
    perm[p_e] = 2 * f
    perm[p_o] = 2 * f + 1
    perm[64 + p_e] = 64 + 2 * f
    perm[64 + p_o] = 64 + 2 * f + 1
    return perm


def _perm(deinterleave_swap):
    one = _perm16()
    if deinterleave_swap:
        # partner layout: swap halves within each quadrant
        p = np.arange(128)
        one = one[(p // 32) * 32 + (p % 32 + 16) % 32]
    return np.concatenate([h * 128 + one for h in range(HPC)])


def _row_tables(sinT, cosT):
    """Expand [32, N] freq tables to the 128-row 16-interleaved layout.
    Returns (cos128, sin128, esign128) where esign is -1 on even rows
    (which receive -sin) and +1 on odd rows."""
    p = np.arange(128)
    q = p % 64
    fidx = (q // 32) * 16 + (q % 16)
    cos128 = cosT[fidx]
    sin128 = sinT[fidx]
    esign = np.where((p % 32) < 16, np.float32(-1.0), np.float32(1.0))
    return cos128, sin128, esign[:, None]


def _prepare_in_maps(context_output, mask, W, b):
    x = np.asarray(context_output, np.float32)
    mask = np.asarray(mask, np.float32)
    W = np.asarray(W, np.float32)
    b = np.asarray(b, np.float32)

    sinT, cosT = _rope_tables()                 # [32, N] fp32
    p1, p2 = _perm(False), _perm(True)
    tril = np.where(np.arange(128)[:, None] > np.arange(128)[None, :],
                    np.float32(-INF8), np.float32(0.0)).astype(np.float32)

    per_g = {}
    for g in range(2):
        Wg = W[g * 768:(g + 1) * 768]
        bg = b[g * 768:(g + 1) * 768]
        w1 = np.ascontiguousarray(Wg[p1].T).reshape(8, 128, 768)
        b1 = np.ascontiguousarray(bg[p1].reshape(HPC, 128).T)
        b2 = np.ascontiguousarray(bg[p2].reshape(HPC, 128).T)
        per_g[g] = (w1, b1, b2)

    cos128, sin128, esign = _row_tables(sinT, cosT)
    in_maps = []
    for c in range(N_CORES):
        bi, g = c // 2, c % 2
        w1, b1, b2 = per_g[g]
        xt = np.ascontiguousarray(x[bi].T).reshape(8, 128, N)
        m = mask[bi]
        mq8 = (m / np.float32(8.0)).astype(np.float32)
        gate = np.concatenate([np.broadcast_to(mq8, (64, N)),
                               np.broadcast_to(m, (64, N))], 0)
        cc = (cos128 * gate).astype(np.float32)
        ss = (sin128 * esign * gate).astype(np.float32)
        im = np.float32(1.0) - m
        mq6 = np.stack([im * np.float32(C1A), im,
                        m * np.float32(C1A), m,
                        np.full(N, C1A, np.float32),
                        np.ones(N, np.float32)]).astype(np.float32)
        nk = np.full(N, np.float32(-C2), np.float32)
        mk6 = np.stack([nk, nk, im * np.float32(-C2), im * np.float32(-C2),
                        nk, nk]).astype(np.float32)
        in_maps.append({
            "xt": xt, "w1": w1, "cc": cc, "ss": ss, "b1": b1, "b2": b2,
            "mq6": mq6, "mk6": mk6, "trl": tril,
        })
    return in_maps


def _get_nc():
    if "nc" not in _CACHE:
        _CACHE["nc"] = _build_program()
    return _CACHE["nc"]


def _get_runner():
    if "runner" not in _CACHE:
        _CACHE["runner"] = _make_runner(_get_nc())
    return _CACHE["runner"]


def _make_runner(nc):
    """Build a jitted shard_map executor over the 8 cores, mirroring
    bass_utils.run_bass_kernel_spmd's axon path but reusable across calls."""
    import jax
    from jax.sharding import Mesh, NamedSharding, PartitionSpec
    from jax.experimental.shard_map import shard_map
    import concourse.bass2jax as bass2jax

    bass2jax.install_neuronx_cc_hook()
    partition_name = (nc.partition_id_tensor.name
                      if nc.partition_id_tensor else None)
    in_names, out_names, out_avals = [], [], []
    for alloc in nc.m.functions[0].allocations:
        if not isinstance(alloc, mybir.MemoryLocationSet):
            continue
        name = alloc.memorylocations[0].name
        if alloc.kind == "ExternalInput":
            if name != partition_name:
                in_names.append(name)
        elif alloc.kind == "ExternalOutput":
            out_names.append(name)
            out_avals.append(jax.core.ShapedArray(
                tuple(alloc.tensor_shape), mybir.dt.np(alloc.dtype)))
    n_params = len(in_names)
    all_names = in_names + out_names
    if partition_name is not None:
        all_names = all_names + [partition_name]

    def _body(*args):
        operands = list(args)
        if partition_name is not None:
            operands.append(bass2jax.partition_id_tensor())
        outs = bass2jax._bass_exec_p.bind(
            *operands, out_avals=tuple(out_avals), in_names=tuple(all_names),
            out_names=tuple(out_names), lowering_input_output_aliases=(),
            sim_require_finite=True, sim_require_nnan=True, nc=nc)
        return tuple(outs)

    devices = jax.devices()[:N_CORES]
    mesh = Mesh(np.asarray(devices), ("core",))
    nin = n_params + len(out_names)
    sharded = jax.jit(
        shard_map(_body, mesh=mesh,
                  in_specs=(PartitionSpec("core"),) * nin,
                  out_specs=(PartitionSpec("core"),) * len(out_names),
                  check_rep=False),
        donate_argnums=tuple(range(n_params, nin)), keep_unused=True)
    shard = NamedSharding(mesh, PartitionSpec("core"))
    zero_shapes = [(N_CORES * a.shape[0], *a.shape[1:]) for a in out_avals]
    return {
        "sharded": sharded, "shard": shard, "in_names": in_names,
        "out_names": out_names, "out_avals": out_avals,
        "zero_shapes": zero_shapes,
    }


def _run(in_maps):
    import jax
    r = _get_runner()
    concat_in = [np.concatenate([in_maps[c][n] for c in range(N_CORES)], 0)
                 for n in r["in_names"]]
    dev_in = [jax.device_put(a, r["shard"]) for a in concat_in]
    zeros = [jax.device_put(np.zeros(s, np.float32), r["shard"])
             for s in r["zero_shapes"]]
    outs = r["sharded"](*dev_in, *zeros)
    return [
        {n: np.asarray(outs[i]).reshape(N_CORES, *r["out_avals"][i].shape)[c]
         for i, n in enumerate(r["out_names"])}
        for c in range(N_CORES)
    ]


def kernel(context_output, mask, W, b):
    in_maps = _prepare_in_maps(context_output, mask, W, b)
    results = _run(in_maps)
    out = np.empty((B, HEADS, N, N), np.float32)
    for c in range(N_CORES):
        bi, g = c // 2, c % 2
        out[bi, g * HPC:(g + 1) * HPC] = results[c]["out"]
    return out
